# revision 1
# baseline (speedup 1.0000x reference)
"""CrossAttention kernel for 8 TRN2 NeuronCores (head-parallel sharding).

Problem: x[2,2048,1024], context[2,2048,1024], 16 heads x 64 dim,
q/k/v projections + softmax attention + output projection.

Sharding: 2 heads per core (e-slice of 128 rows of Wq/Wk/Wv, 128 cols of Wo).
Each core computes a full-shape partial of the output projection for its
heads; the host sums the 8 partials and adds the bias.

All matmuls run as float32r (TF32-like, ~1e-4 rel err, full PE rate).
Layouts are chosen so no on-device transpose of x/context is needed:
the host passes x^T / context^T / W^T, and attention is computed as
S^T tiles ([key,query] layout) so softmax-sum folds into the PE via an
appended ones-column on V.
"""
import sys

sys.path.insert(0, "/opt/trn_rl_repo")

import numpy as np
from contextlib import ExitStack

import concourse.bass as bass  # noqa: F401
import concourse.tile as tile
from concourse import bacc, mybir
from concourse.bass_utils import run_bass_kernel_spmd
from concourse.masks import make_identity

B, N, M = 2, 2048, 2048
QDIM = 1024
HEADS = 16
DH = 64
INNER = 1024
NCORES = 8
ES = INNER // NCORES        # 128: e-slice (2 heads * 64) per core
SCALE = DH ** -0.5
T = B * N                   # 4096 query tokens; key tokens likewise B*M
KC = QDIM // 128            # 8 contraction chunks for the projections
MT = M // 128               # 16 key tiles per batch
F32 = mybir.dt.float32
F32R = mybir.dt.float32r
EXP = mybir.ActivationFunctionType.Exp


def build_nc(reps: int = 1):
    nc = bacc.Bacc("TRN2", target_bir_lowering=False, debug=False,
                   num_devices=NCORES)
    xT = nc.dram_tensor("xT", [QDIM, T], F32, kind="ExternalInput").ap()
    cT = nc.dram_tensor("cT", [QDIM, T], F32, kind="ExternalInput").ap()
    wqT = nc.dram_tensor("wqT", [QDIM, ES], F32, kind="ExternalInput").ap()
    wkT = nc.dram_tensor("wkT", [QDIM, ES], F32, kind="ExternalInput").ap()
    wvT = nc.dram_tensor("wvT", [QDIM, ES], F32, kind="ExternalInput").ap()
    woT = nc.dram_tensor("woT", [ES, QDIM], F32, kind="ExternalInput").ap()
    part = nc.dram_tensor("part", [T, QDIM], F32, kind="ExternalOutput").ap()

    xT3 = xT.rearrange("(kc p) n -> kc p n", p=128)
    cT3 = cT.rearrange("(kc p) n -> kc p n", p=128)

    with tile.TileContext(nc) as tc, ExitStack() as ctx:
        const = ctx.enter_context(tc.tile_pool(name="const", bufs=1))
        big = ctx.enter_context(tc.tile_pool(name="bigsb", bufs=1))
        xsl = ctx.enter_context(tc.tile_pool(name="xsl", bufs=4))
        epool = ctx.enter_context(tc.tile_pool(name="epool", bufs=4))
        opool = ctx.enter_context(tc.tile_pool(name="opool", bufs=2))
        bcp = ctx.enter_context(tc.tile_pool(name="bcp", bufs=2))
        outp = ctx.enter_context(tc.tile_pool(name="outp", bufs=3))
        psB = ctx.enter_context(tc.tile_pool(name="psB", bufs=2, space="PSUM"))
        psA = ctx.enter_context(tc.tile_pool(name="psA", bufs=2, space="PSUM"))

        ident_f = const.tile([128, 128], F32)
        make_identity(nc, ident_f[:])
        ident = const.tile([128, 128], F32R)
        nc.vector.tensor_copy(ident[:], ident_f[:])
        ones_f = const.tile([128, B * MT], F32)
        nc.vector.memset(ones_f[:], 1.0)
        onesR = const.tile([128, B * MT], F32R)
        nc.vector.tensor_copy(onesR[:], ones_f[:])
        wq_sb = const.tile([128, KC, ES], F32R)
        wk_sb = const.tile([128, KC, ES], F32R)
        wv_sb = const.tile([128, KC, ES], F32R)
        wo_sb = const.tile([128, QDIM], F32R)
        nc.gpsimd.dma_start(wq_sb[:], wqT.rearrange("(kc p) e -> p kc e", p=128))
        nc.gpsimd.dma_start(wk_sb[:], wkT.rearrange("(kc p) e -> p kc e", p=128))
        nc.gpsimd.dma_start(wv_sb[:], wvT.rearrange("(kc p) e -> p kc e", p=128))
        nc.gpsimd.dma_start(wo_sb[:], woT)

        for _rep in range(reps):
            QT = big.tile([128, T], F32R, tag="QT")
            KT = big.tile([128, T], F32R, tag="KT")
            VT = big.tile([128, T], F32R, tag="VT")
            vgA = big.tile([128, B * MT, DH + 1], F32R, tag="vgA")
            vgB = big.tile([128, B * MT, DH + 1], F32R, tag="vgB")
            nc.vector.tensor_copy(vgA[:, :, DH], onesR[:])
            nc.vector.tensor_copy(vgB[:, :, DH], onesR[:])
            ocats = {}

            def emit_qproj(b, nbp):
                col0 = b * N + nbp * 1024
                ps = psB.tile([128, 1024], F32, tag="ps")
                for k in range(KC):
                    xs = xsl.tile([128, 1024], F32R, tag="xs")
                    nc.gpsimd.dma_start(xs[:], xT3[k, :, col0:col0 + 1024])
                    for h in range(2):
                        sl = slice(h * 512, (h + 1) * 512)
                        nc.tensor.matmul(ps[:, sl], wq_sb[:, k, :], xs[:, sl],
                                         start=(k == 0), stop=(k == KC - 1))
                nc.vector.tensor_copy(QT[:, col0:col0 + 1024], ps[:])

            def emit_kvproj(b, nbp):
                col0 = b * N + nbp * 1024
                psk = psB.tile([128, 1024], F32, tag="ps")
                psv = psA.tile([128, 1024], F32, tag="pa")
                for k in range(KC):
                    cs = xsl.tile([128, 1024], F32R, tag="xs")
                    nc.gpsimd.dma_start(cs[:], cT3[k, :, col0:col0 + 1024])
                    for h in range(2):
                        sl = slice(h * 512, (h + 1) * 512)
                        nc.tensor.matmul(psk[:, sl], wk_sb[:, k, :], cs[:, sl],
                                         start=(k == 0), stop=(k == KC - 1))
                        nc.tensor.matmul(psv[:, sl], wv_sb[:, k, :], cs[:, sl],
                                         start=(k == 0), stop=(k == KC - 1))
                nc.vector.tensor_copy(KT[:, col0:col0 + 1024], psk[:])
                nc.vector.tensor_copy(VT[:, col0:col0 + 1024], psv[:])

            def emit_vtr(b, mt0, mt1):
                for mt in range(mt0, mt1):
                    g = b * MT + mt
                    mcol = b * N + mt * 128
                    for vg, base in ((vgA, 0), (vgB, DH)):
                        pt = psB.tile([128, DH], F32R, tag="ps")
                        nc.tensor.transpose(
                            pt[:], VT[base:base + DH, mcol:mcol + 128],
                            ident[base:base + DH, base:base + DH])
                        nc.vector.tensor_copy(vg[:, g, 0:DH], pt[:])

            def emit_attn(b, nhf):
                if b not in ocats:
                    ocats[b] = opool.tile([128, N], F32R, tag="oc",
                                          name=f"ocat_b{b}_{_rep}")
                ocat = ocats[b]
                qcol = b * N + nhf * 1024
                oA = psA.tile([128, 1024], F32, tag="pa")
                oB = psA.tile([128, 1024], F32, tag="pa")
                for mc in range(MT):
                    g = b * MT + mc
                    mcol = b * N + mc * 128
                    stA = psB.tile([128, 1024], F32, tag="ps")
                    stB = psB.tile([128, 1024], F32, tag="ps")
                    for nb in range(2):
                        sl = slice(nb * 512, (nb + 1) * 512)
                        qsl = slice(qcol + nb * 512, qcol + (nb + 1) * 512)
                        nc.tensor.matmul(stA[:, sl], KT[0:DH, mcol:mcol + 128],
                                         QT[0:DH, qsl], start=True, stop=True)
                        nc.tensor.matmul(stB[:, sl],
                                         KT[DH:2 * DH, mcol:mcol + 128],
                                         QT[DH:2 * DH, qsl],
                                         start=True, stop=True)
                    eA = epool.tile([128, 1024], F32R, tag="e")
                    eB = epool.tile([128, 1024], F32R, tag="e")
                    nc.scalar.activation(eA[:], stA[:], EXP, scale=SCALE)
                    nc.scalar.activation(eB[:], stB[:], EXP, scale=SCALE)
                    last = (mc == MT - 1)
                    for nb in range(2):
                        sl = slice(nb * 512, (nb + 1) * 512)
                        nc.tensor.matmul(oA[0:DH + 1, sl], vgA[:, g, :],
                                         eA[:, sl], start=(mc == 0), stop=last)
                        nc.tensor.matmul(oB[0:DH + 1, sl], vgB[:, g, :],
                                         eB[:, sl], start=(mc == 0), stop=last)
                for o_ps, row0 in ((oA, 0), (oB, DH)):
                    rr = bcp.tile([1, 1024], F32, tag="rr")
                    nc.vector.reciprocal(rr[:], o_ps[DH:DH + 1, 0:1024])
                    bc = bcp.tile([DH, 1024], F32, tag="bc")
                    nc.gpsimd.partition_broadcast(bc[:], rr[:])
                    nc.vector.tensor_mul(
                        ocat[row0:row0 + DH, nhf * 1024:(nhf + 1) * 1024],
                        o_ps[0:DH, 0:1024], bc[:])

            def emit_wo(b):
                ocat = ocats[b]
                for nt in range(N // 128):
                    po = psB.tile([128, 1024], F32, tag="ps")
                    for ob in range(2):
                        nc.tensor.matmul(po[:, ob * 512:(ob + 1) * 512],
                                         ocat[:, nt * 128:(nt + 1) * 128],
                                         wo_sb[:, ob * 512:(ob + 1) * 512],
                                         start=True, stop=True)
                    osb = outp.tile([128, 1024], F32, tag="os")
                    nc.vector.tensor_copy(osb[:], po[:])
                    nc.sync.dma_start(
                        part[b * N + nt * 128:b * N + (nt + 1) * 128, :], osb[:])

            # emission order chosen so attention(b=0) starts as soon as the
            # first K/V blocks land, and batch-1 projections / Wo(0) fill the
            # PE+DMA slack inside the ACT-paced attention stream.
            for b in range(B):
                emit_qproj(b, 0)
                emit_qproj(b, 1)
                emit_kvproj(b, 0)
                emit_kvproj(b, 1)
                emit_vtr(b, 0, MT)
                emit_attn(b, 0)
                emit_attn(b, 1)
                emit_wo(b)
    nc.compile()
    return nc


def make_in_maps(x, context, Wq, Wk, Wv, Wo):
    x = np.asarray(x, dtype=np.float32)
    context = np.asarray(context, dtype=np.float32)
    Wq = np.asarray(Wq, dtype=np.float32)
    Wk = np.asarray(Wk, dtype=np.float32)
    Wv = np.asarray(Wv, dtype=np.float32)
    Wo = np.asarray(Wo, dtype=np.float32)
    xT = np.ascontiguousarray(x.reshape(T, QDIM).T)
    cT = np.ascontiguousarray(context.reshape(T, QDIM).T)
    in_maps = []
    for c in range(NCORES):
        es = slice(c * ES, (c + 1) * ES)
        in_maps.append({
            "xT": xT,
            "cT": cT,
            "wqT": np.ascontiguousarray(Wq[es, :].T),
            "wkT": np.ascontiguousarray(Wk[es, :].T),
            "wvT": np.ascontiguousarray(Wv[es, :].T),
            "woT": np.ascontiguousarray(Wo[:, es].T),
        })
    return in_maps


_NC_CACHE = {}


def get_nc(reps: int = 1):
    if reps not in _NC_CACHE:
        _NC_CACHE[reps] = build_nc(reps)
    return _NC_CACHE[reps]


def run_on_hw(in_maps, reps: int = 1):
    nc = get_nc(reps)
    return run_bass_kernel_spmd(nc, in_maps, core_ids=list(range(NCORES)))


def kernel(x, context, Wq, Wk, Wv, Wo, bo):
    in_maps = make_in_maps(x, context, Wq, Wk, Wv, Wo)
    res = run_on_hw(in_maps, reps=1)
    acc = res.results[0]["part"].astype(np.float32).copy()
    for i in range(1, NCORES):
        acc += res.results[i]["part"]
    acc += np.asarray(bo, dtype=np.float32)[None, :]
    return acc.reshape(B, N, QDIM)



# revision 2
# speedup vs baseline: 3.0737x; 3.0737x over previous
"""CrossAttention kernel for 8 TRN2 NeuronCores — v2 (bf16, interleaved).

Head-parallel sharding (2 heads/core). All-bf16 data path: x/context/weights
arrive as bf16, matmuls run bf16 (1 col/cycle), PSUM accumulation fp32.
Partial outputs are written bf16 and summed (fp32) on the host with the bias.

Structure: 8 attention phases (batch x query-half x head), each ACT-paced by
the softmax exp stream. Projection / Wo / transpose work for other
batches/phases is woven between attention steps so the PE and DMA stay busy
inside the ACT-bound phases. S matmuls are emitted as even/odd key-tile pairs
at partition bases 0/64 so they run concurrently in separate PE row groups
(K=64 row tiling); K and Q are duplicated across both partition halves to
enable this.

Engine routing: loads on SP (sync) via HWDGE, stores + their staging copies
on DVE, projection copies / broadcasts on GpSimd, exp on ACT.
"""
import sys

sys.path.insert(0, "/opt/trn_rl_repo")

import numpy as np
import ml_dtypes
from contextlib import ExitStack

import concourse.bass as bass  # noqa: F401
import concourse.tile as tile
from concourse import bacc, mybir
from concourse.bass_utils import run_bass_kernel_spmd
from concourse.masks import make_identity

B, N, M = 2, 2048, 2048
QDIM = 1024
HEADS = 16
DH = 64
INNER = 1024
NCORES = 8
ES = INNER // NCORES        # 128: e-slice (2 heads * 64) per core
SCALE = DH ** -0.5
T = B * N                   # 4096 tokens total
KC = QDIM // 128            # 8 contraction chunks for projections
MT = M // 128               # 16 key tiles per batch
F32 = mybir.dt.float32
BF = mybir.dt.bfloat16
EXP = mybir.ActivationFunctionType.Exp
BF_NP = ml_dtypes.bfloat16


def build_nc(reps: int = 1):
    nc = bacc.Bacc("TRN2", target_bir_lowering=False, debug=False,
                   num_devices=NCORES)
    xT = nc.dram_tensor("xT", [QDIM, T], BF, kind="ExternalInput").ap()
    cT = nc.dram_tensor("cT", [QDIM, T], BF, kind="ExternalInput").ap()
    wqT = nc.dram_tensor("wqT", [QDIM, ES], BF, kind="ExternalInput").ap()
    wkT = nc.dram_tensor("wkT", [QDIM, ES], BF, kind="ExternalInput").ap()
    wvT = nc.dram_tensor("wvT", [QDIM, ES], BF, kind="ExternalInput").ap()
    woT = nc.dram_tensor("woT", [ES, QDIM], BF, kind="ExternalInput").ap()
    part = nc.dram_tensor("part", [T, QDIM], BF, kind="ExternalOutput").ap()

    xT3 = xT.rearrange("(kc p) n -> kc p n", p=128)
    cT3 = cT.rearrange("(kc p) n -> kc p n", p=128)

    with tile.TileContext(nc) as tc, ExitStack() as ctx:
        const = ctx.enter_context(tc.tile_pool(name="const", bufs=1))
        big = ctx.enter_context(tc.tile_pool(name="bigsb", bufs=1))
        xsl = ctx.enter_context(tc.tile_pool(name="xsl", bufs=1))
        epool = ctx.enter_context(tc.tile_pool(name="epool", bufs=4))
        bcp = ctx.enter_context(tc.tile_pool(name="bcp", bufs=1))
        outp = ctx.enter_context(tc.tile_pool(name="outp", bufs=3))
        ps_st = ctx.enter_context(tc.tile_pool(name="ps_st", bufs=2, space="PSUM"))
        ps_o = ctx.enter_context(tc.tile_pool(name="ps_o", bufs=1, space="PSUM"))
        pp = ctx.enter_context(tc.tile_pool(name="pp", bufs=2, space="PSUM"))

        # warm the exp table before the first real activation
        warm = const.tile([1, 16], F32)
        nc.vector.memset(warm[:], 0.0)
        warm_o = const.tile([1, 16], F32)
        nc.scalar.activation(warm_o[:], warm[:], EXP)
        wq_sb = const.tile([128, KC, ES], BF)
        wk_sb = const.tile([128, KC, ES], BF)
        wv_sb = const.tile([128, KC, ES], BF)
        wo_sb = const.tile([128, QDIM], BF)
        # K weights first: the first projection matmul waits on them
        nc.sync.dma_start(wk_sb[:], wkT.rearrange("(kc p) e -> p kc e", p=128))
        nc.sync.dma_start(wv_sb[:], wvT.rearrange("(kc p) e -> p kc e", p=128))
        nc.sync.dma_start(wq_sb[:], wqT.rearrange("(kc p) e -> p kc e", p=128))
        nc.sync.dma_start(wo_sb[:], woT)

        for _rep in range(reps):
            # per-(b,head) tensors; rows 0:64 and 64:128 hold the same head
            # (dup) so S matmul pairs can row-tile at bases 0/64.
            Qd = {}
            Kd = {}
            vg = {}
            VT = {}
            oc = {}
            for b in range(B):
                VT[b] = big.tile([128, N], BF, tag=f"vt{b}", name=f"VT{b}_{_rep}")
                oc[b] = big.tile([128, N], BF, tag=f"oc{b}", name=f"oc{b}_{_rep}")
                for h in range(2):
                    Qd[b, h] = big.tile([128, N], BF, tag=f"qd{b}{h}",
                                        name=f"Qd{b}{h}_{_rep}")
                    Kd[b, h] = big.tile([128, N], BF, tag=f"kd{b}{h}",
                                        name=f"Kd{b}{h}_{_rep}")
                    v = big.tile([128, MT, DH + 1], BF, tag=f"vg{b}{h}",
                                 name=f"vg{b}{h}_{_rep}")
                    nc.vector.memset(v[:, :, DH], 1.0)
                    vg[b, h] = v

            # ---- work units ----------------------------------------------
            def u_load_group(src3, b, g):
                """DMA 8 contraction chunks of 1024 tokens into SBUF."""
                col0 = b * N + g * 1024
                xs = []
                for k in range(KC):
                    t = xsl.tile([128, 1024], BF, tag="xs", bufs=20,
                                 name=f"xs_{_rep}_{b}_{g}_{k}")
                    nc.sync.dma_start(t[:], src3[k, :, col0:col0 + 1024])
                    xs.append(t)
                return xs

            def u_proj_sub(xs, w_sb, half, dsts):
                """One 512-token projection accumulation + copies.

                dsts: list of (dst_ap, src_rows) writes from the psum tile,
                alternated across GpSimd/DVE to balance copy load.
                """
                ps = pp.tile([128, 512], F32, tag="x", name=f"ps_{_rep}")
                sl = slice(half * 512, (half + 1) * 512)
                for k in range(KC):
                    nc.tensor.matmul(ps[:], w_sb[:, k, :], xs[k][:, sl],
                                     start=(k == 0), stop=(k == KC - 1))
                for i, (dst, rows) in enumerate(dsts):
                    eng = nc.gpsimd if i % 2 == 0 else nc.vector
                    eng.tensor_copy(dst, ps[rows, :])

            def u_q_sub(b, g, half, xs):
                col = slice(g * 1024 + half * 512, g * 1024 + (half + 1) * 512)
                u_proj_sub(xs, wq_sb, half, [
                    (Qd[b, 0][0:64, col], slice(0, 64)),
                    (Qd[b, 0][64:128, col], slice(0, 64)),
                    (Qd[b, 1][0:64, col], slice(64, 128)),
                    (Qd[b, 1][64:128, col], slice(64, 128)),
                ])

            def u_k_sub(b, g, half, xs):
                col = slice(g * 1024 + half * 512, g * 1024 + (half + 1) * 512)
                u_proj_sub(xs, wk_sb, half, [
                    (Kd[b, 0][0:64, col], slice(0, 64)),
                    (Kd[b, 0][64:128, col], slice(0, 64)),
                    (Kd[b, 1][0:64, col], slice(64, 128)),
                    (Kd[b, 1][64:128, col], slice(64, 128)),
                ])

            def u_v_sub(b, g, half, xs):
                col = slice(g * 1024 + half * 512, g * 1024 + (half + 1) * 512)
                u_proj_sub(xs, wv_sb, half, [(VT[b][:, col], slice(0, 128))])

            def u_vtr(b, g):
                """Transpose one 128-key tile of V into vg via XBAR DMA."""
                for h in range(2):
                    nc.sync.dma_start_transpose(
                        vg[b, h][:, g, 0:DH],
                        VT[b][h * DH:(h + 1) * DH, g * 128:(g + 1) * 128])

            def u_wo(b, nt):
                """Output projection for one 128-token tile + store."""
                osb = outp.tile([128, QDIM], BF, tag="os", name=f"os_{_rep}")
                otile = oc[b][:, nt * 128:(nt + 1) * 128]
                for ob in range(2):
                    po = pp.tile([128, 512], F32, tag="x", name=f"po_{_rep}")
                    nc.tensor.matmul(po[:], otile, wo_sb[:, ob * 512:(ob + 1) * 512],
                                     start=True, stop=True)
                    nc.vector.tensor_copy(osb[:, ob * 512:(ob + 1) * 512], po[:])
                r0 = b * N + nt * 128
                nc.gpsimd.dma_start(part[r0:r0 + 128, :], osb[:])

            # ---- attention phase -----------------------------------------
            def attn_phase(b, nhf, h, drain):
                """16 key tiles for (batch, query-half, head); drain() is
                called once per pair step to weave in side work."""
                q0 = nhf * 1024
                oacc = ps_o.tile([128, 1024], F32, tag="o", name=f"oacc_{_rep}")
                es = {}

                def s_mm(mc):
                    rs = slice(0, 64) if mc % 2 == 0 else slice(64, 128)
                    st = ps_st.tile([128, 1024], F32, tag="st",
                                    name=f"st_{_rep}")
                    for qb in range(2):
                        nc.tensor.matmul(
                            st[:, qb * 512:(qb + 1) * 512],
                            Kd[b, h][rs, mc * 128:(mc + 1) * 128],
                            Qd[b, h][rs, q0 + qb * 512:q0 + (qb + 1) * 512],
                            start=True, stop=True)
                    e = epool.tile([128, 1024], BF, tag="e", name=f"e_{_rep}")
                    nc.scalar.activation(e[:], st[:], EXP, scale=SCALE)
                    es[mc] = e

                def av_mm(mc):
                    e = es.pop(mc)
                    for qb in range(2):
                        nc.tensor.matmul(
                            oacc[0:DH + 1, qb * 512:(qb + 1) * 512],
                            vg[b, h][:, mc, :], e[:, qb * 512:(qb + 1) * 512],
                            start=(mc == 0), stop=(mc == MT - 1))

                for p in range(MT // 2):
                    s_mm(2 * p)
                    s_mm(2 * p + 1)
                    if p > 0:
                        av_mm(2 * p - 2)
                        av_mm(2 * p - 1)
                    drain()
                av_mm(MT - 2)
                av_mm(MT - 1)

                rr = bcp.tile([1, 1024], F32, tag="rr", name=f"rr_{_rep}")
                nc.vector.reciprocal(rr[:], oacc[DH:DH + 1, 0:1024])
                bc = bcp.tile([DH, 1024], F32, tag="bc", name=f"bc_{_rep}")
                nc.gpsimd.partition_broadcast(bc[:], rr[:])
                r0 = h * DH
                nc.vector.tensor_mul(
                    oc[b][r0:r0 + DH, q0:q0 + 1024],
                    oacc[0:DH, 0:1024], bc[:])

            # ---- emission schedule ---------------------------------------
            # Global work queue: (avail_phase, deadline_phase, cost_ns, fn).
            # At each pair step, overdue units (deadline <= current phase)
            # are emitted unconditionally; otherwise available units are
            # emitted while the accumulated side debt stays under the
            # ACT-slack budget.
            def kv_units(b, g):
                xs = u_load_group(cT3, b, g)
                return [
                    lambda: u_k_sub(b, g, 0, xs),
                    lambda: u_k_sub(b, g, 1, xs),
                    lambda: u_v_sub(b, g, 0, xs),
                    lambda: u_v_sub(b, g, 1, xs),
                ]

            def q_units(b, g):
                xs = u_load_group(xT3, b, g)
                return [
                    lambda: u_q_sub(b, g, 0, xs),
                    lambda: u_q_sub(b, g, 1, xs),
                ]

            # prologue: first half of batch-0 K/V + vg, first Q quarter.
            # vtr DMAs go last so they don't head-of-line-block the x loads
            # on the SP queue while waiting for the V copies.
            for u in kv_units(0, 0):
                u()
            for u in q_units(0, 0):
                u()
            for g in range(8):
                u_vtr(0, g)

            work = []

            def add(avail, deadline, cost, fn):
                """avail/deadline in global pair steps (8 phases x 8 pairs).

                A unit is force-emitted at its deadline pair (its consumer
                instruction is emitted ~2 pairs later, so producers always
                precede consumers in the engine FIFOs — deadlock-free);
                earlier emission happens when the ACT-slack budget allows.
                """
                work.append([avail, deadline, cost, fn])

            kv01 = kv_units(0, 1)
            add(0, 1, 1700, kv01[2])   # V covering key tiles 8..11
            add(0, 2, 1700, kv01[0])   # K 8..11 (S(8) at pair 4)
            add(0, 3, 1700, kv01[3])   # V 12..15
            add(0, 4, 1700, kv01[1])   # K 12..15 (S(12) at pair 6)
            for g in range(8, 16):
                add(0, 1 + g // 2, 100, lambda g=g: u_vtr(0, g))
            for u in q_units(0, 1):
                add(0, 13, 1700, u)    # P2 starts at pair 16
            kv10 = kv_units(1, 0)
            kv11 = kv_units(1, 1)
            add(8, 20, 1700, kv10[2])
            add(8, 22, 1700, kv10[0])
            add(8, 21, 1700, kv10[3])
            add(8, 23, 1700, kv10[1])
            add(10, 24, 1700, kv11[2])
            add(10, 26, 1700, kv11[0])
            add(10, 25, 1700, kv11[3])
            add(10, 27, 1700, kv11[1])
            for g in range(8):
                add(16, 23, 100, lambda g=g: u_vtr(1, g))
            for g in range(8, 16):
                add(16, 27, 100, lambda g=g: u_vtr(1, g))
            q10 = q_units(1, 0)
            add(16, 25, 1700, q10[0])   # P4 starts at pair 32
            add(16, 26, 1700, q10[1])
            for u in q_units(1, 1):
                add(24, 45, 1700, u)   # P6 starts at pair 48
            for nt in range(8):
                add(16, 62, 860, lambda nt=nt: u_wo(0, nt))
            for nt in range(8, 16):
                add(32, 62, 860, lambda nt=nt: u_wo(0, nt))
            for nt in range(8):
                add(48, 62, 860, lambda nt=nt: u_wo(1, nt))
            for nt in range(8, 16):
                add(64, 999, 860, lambda nt=nt: u_wo(1, nt))

            SLACK_NS = 520.0
            state = {"pair": 0, "budget": 0.0, "debt": 0.0}

            def drain():
                pr = state["pair"]
                state["pair"] += 1
                state["budget"] += SLACK_NS
                while True:
                    pick = None
                    for it in work:
                        if it[1] <= pr:
                            pick = it
                            break
                    if pick is None and state["debt"] < state["budget"]:
                        for it in work:
                            if it[0] <= pr:
                                pick = it
                                break
                    if pick is None:
                        return
                    work.remove(pick)
                    state["debt"] += pick[2]
                    pick[3]()
                    if pick[1] > pr and state["debt"] >= state["budget"]:
                        return

            for b in range(B):
                for nhf in range(2):
                    for h in range(2):
                        attn_phase(b, nhf, h, drain)
            for it in work:
                it[3]()
    nc.compile()
    return nc


def make_in_maps(x, context, Wq, Wk, Wv, Wo):
    x = np.asarray(x, dtype=np.float32)
    context = np.asarray(context, dtype=np.float32)
    Wq = np.asarray(Wq, dtype=np.float32)
    Wk = np.asarray(Wk, dtype=np.float32)
    Wv = np.asarray(Wv, dtype=np.float32)
    Wo = np.asarray(Wo, dtype=np.float32)
    xT = np.ascontiguousarray(x.reshape(T, QDIM).T).astype(BF_NP)
    cT = np.ascontiguousarray(context.reshape(T, QDIM).T).astype(BF_NP)
    in_maps = []
    for c in range(NCORES):
        es = slice(c * ES, (c + 1) * ES)
        in_maps.append({
            "xT": xT,
            "cT": cT,
            "wqT": np.ascontiguousarray(Wq[es, :].T).astype(BF_NP),
            "wkT": np.ascontiguousarray(Wk[es, :].T).astype(BF_NP),
            "wvT": np.ascontiguousarray(Wv[es, :].T).astype(BF_NP),
            "woT": np.ascontiguousarray(Wo[:, es].T).astype(BF_NP),
        })
    return in_maps


_NC_CACHE = {}


def get_nc(reps: int = 1):
    if reps not in _NC_CACHE:
        _NC_CACHE[reps] = build_nc(reps)
    return _NC_CACHE[reps]


def run_on_hw(in_maps, reps: int = 1):
    nc = get_nc(reps)
    return run_bass_kernel_spmd(nc, in_maps, core_ids=list(range(NCORES)))


def kernel(x, context, Wq, Wk, Wv, Wo, bo):
    in_maps = make_in_maps(x, context, Wq, Wk, Wv, Wo)
    res = run_on_hw(in_maps, reps=1)
    acc = res.results[0]["part"].astype(np.float32)
    for i in range(1, NCORES):
        acc = acc + res.results[i]["part"].astype(np.float32)
    acc += np.asarray(bo, dtype=np.float32)[None, :]
    return acc.reshape(B, N, QDIM)
